# revision 59
# baseline (speedup 1.0000x reference)
"""TRN2 Bass kernel for nn_Attention_41506563948971.

Reference computation (per batch b):
    G  = (q @ w + b) @ a^T          [Lq, La]
    P  = softmax(G, axis=q)         (softmax over dim=1, the q axis)
    out= P^T @ q                    [La, H]

Sharding: data-parallel over batch B=8 across the 8 NeuronCores; w, b
replicated. Each core computes one full batch; no collectives.

Numerics: logits G have sigma ~= 1024 (q,a ~ N(0,1), H=1024), so the dim-q
softmax is peaked and logit errors on near-max entries turn into output
errors. MM1/MM2 run as single-pass float32r matmuls: the PE reads 4-byte
fp32 operands rounded to ~FP22 (13-bit mantissa) at the full 1-cycle/row
rate (4x the true-fp32 rate), giving logit abs err ~5e-3 - no hi/lo split
passes needed (the fp16 hi/lo 3-pass scheme costs 3x the PE time for
precision the 2e-2 gate does not need; measured end-to-end rel err is
6.4e-3 vs the baseline split kernel's 2.1e-4). The BIR verifier requires
every fp32r matmul operand to be produced by an instruction that rounds
to fp32r, so the w/qT/QwT/aT tiles are written with float32r output dtype
by the DVE/ACT ops that stage them. MM3's operands are one-hot-ish
softmax weights and q, where 11-bit fp16 rounding gives ~2e-4 relative
error at full PE speed. The softmax normalization (1/sum) is folded into
a per-partition scale on the small MM3 output, so the big exp matrix is
never divided.

Schedule notes (cost-model span ~281 us/core, PE busy ~268 us at 95%):
- ~28 warmup matmuls pre-ramp the HAM clock gate so the real matmuls
  start at 2.4 GHz, not 1.2 GHz. Any PE idle gap resets the ramp (the
  next ~3 us of matmuls then run at 0.65-1.2 GHz), so the PE stream is
  kept gapless: plain back-to-back matmul groups with deep PSUM pools.
- q^T and a^T are produced by PE transposes (fp32, 2 cycles/row, batched
  8 per PSUM region) with a single strided DVE evacuation that also
  performs the fp32r rounding; the xbar DMA transpose engine only handles
  2-byte dtypes so it is reserved for the fp16 E^T transpose. (A
  Pool-engine fp16-pair split + xbar transpose + recombine variant frees
  13.7 us of PE but its cross-engine chains perturb the list scheduler
  into SEQ stalls that cost more than they save.)
- DmaTransposeAnt must be issued on the ACT hwdge queue: on the SP queue
  it produces wrong results (races) on the NEFF/fake_nrt path.
- MM3 runs as fp8-e4m3 DoubleRow (q stored as an e4m3 hi/lo pair,
  [P,2,n] APs contracting two q-tiles per matmul at 0.5 cycles/row =
  half the fp16 cost; out rel err +1e-4). E^T is PE-transposed in fp8
  (1 cy/row; output element step must be 2, hence the [*,2]-padded PSUM
  tile) so the transposes ride the PE stream right behind MM2 — an
  xbar-transpose + DVE-cast chain instead blocks the in-order PE
  sequencer at the MM3 Ldweights and costs +10..70 us.
- bias-add + QwT evacuation runs on ScalarE as an Identity activation
  with AP bias and float32r output dtype.
- MM2 runs nq-outer so each GT chunk's reduce_max overlaps the next
  chunk's matmuls; exps are emitted ahead of MM3's scales on ACT's
  in-order queue; MM3 is software-pipelined one a-tile behind so PE has
  work while ACT runs the exps.
- a-tiles 0/1 are transposed at MM1 chunk boundaries from a persistent
  stage (phase-2 pools reuse phase-1 SBUF, so phase-2-staged tiles
  cannot even load until MM1 ends); a second warmup block bridges the
  startup window between the chunk-0 transposes and the last w tile;
  the final iteration transposes/evacuates E^T per 512-chunk and each
  MM3 output half is stored as soon as it is scaled, shortening the
  serial tail.
"""

import sys

sys.path.insert(0, "/opt/trn_rl_repo")

from contextlib import ExitStack

import numpy as np

import concourse.bass as bass
import concourse.bacc as bacc
import concourse.mybir as mybir
import concourse.tile as tile
from concourse.masks import make_identity

dt = mybir.dt
AF = mybir.ActivationFunctionType
OP = mybir.AluOpType
AX = mybir.AxisListType

P = 128
H = 1024
KO = H // P          # 8 contraction chunks
LQ = 2048
LA = 2048
NQT = LQ // P        # 16 q row-tiles
NAT = LA // P        # 16 a row-tiles
QC = 512             # free-dim chunk (one fp32 PSUM bank)
NQC = LQ // QC       # 4
B = 8                # batch == number of cores


def _trace_kernel(tc, q_d, a_d, w_d, b_d, o_d):
    nc = tc.nc
    with ExitStack() as ctx:
        pp = ctx.enter_context(tc.tile_pool(name="persist", bufs=1))

        id_f32 = pp.tile([P, P], dt.float32, tag="id_f32")
        make_identity(nc, id_f32[:])

        # PE clock warmup (HAM gate holds PE at 1.2 GHz until ~3.4 us of
        # sustained activity; PE would idle waiting for the first loads).
        warm_sb = pp.tile([P, P], dt.float16, tag="warm_sb")
        nc.vector.memset(warm_sb[:], 1.0)

        b_sb = pp.tile([P, KO], dt.float32, tag="b_sb")

        # QwT = (q @ w + b)^T in [h, q] layout, fp32r (PE reads at full rate).
        qwt_r = pp.tile([P, KO, LQ], dt.float32r, tag="qwt_r")
        # q in natural [q, h] layout as an e4m3 hi/lo pair for the fp8
        # DoubleRow MM3 (2 passes at 0.5 cycles/row = half the fp16 cost).
        q8h = pp.tile([P, NQT, H], dt.float8e4, tag="q8h")
        q8l = pp.tile([P, NQT, H], dt.float8e4, tag="q8l")
        id_f8 = pp.tile([P, P], dt.float8e4, tag="id_f8")
        make_identity(nc, id_f8[:])
        id_fr = pp.tile([P, P], dt.float32r, tag="id_fr")
        nc.vector.tensor_copy(id_fr[:], id_f32[:])
        # a-tiles 0/1 prepped during MM1 (phase-2 pools reuse phase-1
        # SBUF, so anything staged there cannot even load until MM1's
        # last matmul releases w_r); the shared stage is reused
        # sequentially, only the fp32r results persist into phase 2
        a01_stage = pp.tile([P, H], dt.float32, tag="a01_stage")
        a0t_r = pp.tile([P, KO, P], dt.float32r, tag="a0t_r")
        a1t_r = pp.tile([P, KO, P], dt.float32r, tag="a1t_r")

        # ---------------- Phase 1: MM1 -> QwT ----------------
        with ExitStack() as p1:
            ps_pool = p1.enter_context(
                tc.tile_pool(name="ps1", bufs=4, space="PSUM"))
            tp_pool = p1.enter_context(
                tc.tile_pool(name="tp1", bufs=2, space="PSUM"))
            wpool = p1.enter_context(tc.tile_pool(name="wpool", bufs=1))
            stage = p1.enter_context(tc.tile_pool(name="stage", bufs=4))
            qtp = p1.enter_context(tc.tile_pool(name="qtp", bufs=2))

            warm_ps = tp_pool.tile([P, P], dt.float32, tag="tp",
                                   name="warm_ps")
            NWARM = 28
            for j in range(NWARM):
                nc.tensor.matmul(
                    warm_ps[:], warm_sb[:], warm_sb[:],
                    start=(j == 0), stop=(j == NWARM - 1),
                )

            w_r = wpool.tile([P, KO, H], dt.float32r, tag="w_r")

            def load_w(k):
                wt = stage.tile([P, H], dt.float32, tag="wstage", name=f"wt{k}")
                nc.sync.dma_start(wt[:], w_d[k * P:(k + 1) * P, :])
                nc.vector.tensor_copy(w_r[:, k], wt[:])

            def prep_q_tile(qc, t, qt_r):
                qs = stage.tile([P, H], dt.float32, tag="qstage",
                                name=f"qs{qc}_{t}")
                row0 = qc * QC + t * P
                nc.sync.dma_start(qs[:], q_d[row0:row0 + P, :])
                tq = qc * (QC // P) + t
                # e4m3 hi/lo pair for MM3: hi cast on ScalarE, residual on
                # VectorE (reads hi back upconverted)
                nc.scalar.copy(q8h[:, tq], qs[:])
                nc.vector.tensor_tensor(q8l[:, tq], qs[:], q8h[:, tq],
                                        OP.subtract)
                # PE transpose, batched 8 per PSUM region, one strided DVE
                # evacuation that also rounds to fp32r
                tp = tp_pool.tile([P, KO * P], dt.float32, tag="tp")
                for k in range(KO):
                    nc.tensor.transpose(
                        tp[:, k * P:(k + 1) * P],
                        qs[:, k * P:(k + 1) * P],
                        id_f32[:],
                    )
                nc.vector.tensor_copy(
                    qt_r[:, :, t * P:(t + 1) * P],
                    tp[:].rearrange("p (k c) -> p k c", k=KO),
                )

            def alloc_qt(qc):
                return qtp.tile([P, KO, QC], dt.float32r, tag="qt_r",
                                name=f"qt{qc}")

            # q-chunk 0's loads/transposes first so PE starts immediately;
            # w loads overlap the transposes.
            def warm_block(n, name):
                wps = ps_pool.tile([P, P], dt.float32, tag="ps", name=name)
                for j in range(n):
                    nc.tensor.matmul(
                        wps[:], warm_sb[:], warm_sb[:],
                        start=(j == 0), stop=(j == n - 1),
                    )

            qt_cur = alloc_qt(0)
            for t in range(QC // P):
                prep_q_tile(0, t, qt_cur)
            # strided 1024-descriptor gather: keep it off the SP queue and
            # behind the startup-critical q loads
            nc.gpsimd.dma_start(b_sb[:], b_d.rearrange("(m p) -> p m", p=P))
            for k in range(KO):
                load_w(k)

            def prep_a_early(i, dst):
                # PE transposes ride the MM1 stream at a chunk boundary;
                # the load was issued earlier so the data is resident
                tp = tp_pool.tile([P, KO * P], dt.float32, tag="tp")
                for k in range(KO):
                    nc.tensor.transpose(
                        tp[:, k * P:(k + 1) * P],
                        a01_stage[:, k * P:(k + 1) * P],
                        id_f32[:],
                    )
                nc.vector.tensor_copy(
                    dst[:], tp[:].rearrange("p (k c) -> p k c", k=KO)
                )

            nc.sync.dma_start(a01_stage[:], a_d[0:P, :])
            # filler block covering the window between the chunk-0
            # transposes and the last w tile's arrival
            warm_block(100, "warm2")
            for qc in range(NQC):
                if qc == 2:
                    prep_a_early(0, a0t_r)
                    nc.sync.dma_start(a01_stage[:], a_d[P:2 * P, :])
                elif qc == 3:
                    prep_a_early(1, a1t_r)
                if qc + 1 < NQC:
                    qt_next = alloc_qt(qc + 1)
                for m in range(KO):
                    acc = ps_pool.tile([P, QC], dt.float32, tag="ps")
                    for k in range(KO):
                        nc.tensor.matmul(
                            acc[:],
                            w_r[:, k, m * P:(m + 1) * P],
                            qt_cur[:, k, :],
                            start=(k == 0),
                            stop=(k == KO - 1),
                        )
                    # bias add + fp32r rounding + evacuation on ScalarE
                    nc.scalar.activation(
                        qwt_r[:, m, qc * QC:(qc + 1) * QC], acc[:],
                        AF.Identity, bias=b_sb[:, m:m + 1],
                    )
                    # interleave the next chunk's per-tile prep between
                    # m-blocks so loads/transposes land just ahead of use
                    if qc + 1 < NQC and m < QC // P:
                        prep_q_tile(qc + 1, m, qt_next)
                if qc + 1 < NQC:
                    qt_cur = qt_next

        # ---------------- Phase 2: MM2 + softmax + MM3 ----------------
        with ExitStack() as p2:
            ps_pool = p2.enter_context(
                tc.tile_pool(name="ps2", bufs=6, space="PSUM"))
            tp_pool = p2.enter_context(
                tc.tile_pool(name="tp2", bufs=1, space="PSUM"))
            astage = p2.enter_context(tc.tile_pool(name="astage", bufs=3))
            atp = p2.enter_context(tc.tile_pool(name="atp", bufs=2))
            ppool = p2.enter_context(tc.tile_pool(name="ppool", bufs=2))
            ptpool = p2.enter_context(tc.tile_pool(name="ptpool", bufs=2))
            outp = p2.enter_context(tc.tile_pool(name="outp", bufs=2))
            redp = p2.enter_context(tc.tile_pool(name="redp", bufs=4))

            def prep_a_tile(i):
                at = astage.tile([P, H], dt.float32, tag="astage",
                                 name=f"at{i}")
                nc.sync.dma_start(at[:], a_d[i * P:(i + 1) * P, :])
                # round to fp32r first (ScalarE, which has slack): the PE
                # transposes then run at 1.5 cycles/row instead of 2.0
                at_rr = astage.tile([P, H], dt.float32r, tag="a_rr",
                                    name=f"atrr{i}")
                nc.scalar.copy(at_rr[:], at[:])
                at_r = atp.tile([P, KO, P], dt.float32r, tag="at_r",
                                name=f"atr{i}")
                tp = tp_pool.tile([P, KO * P], dt.float32r, tag="tp")
                for k in range(KO):
                    nc.tensor.transpose(
                        tp[:, k * P:(k + 1) * P],
                        at_rr[:, k * P:(k + 1) * P],
                        id_fr[:],
                    )
                nc.vector.tensor_copy(
                    at_r[:], tp[:].rearrange("p (k c) -> p k c", k=KO)
                )
                return at_r

            def do_mm3(pt8, rinv, i):
                # MM3: out[a, h] = sum_q ET[q, a] * q[q, h], then * (1/sum).
                # fp8 e4m3 DoubleRow: each matmul contracts a PAIR of
                # q-tiles ([P, 2, n] APs) at 0.5 cycles/row; hi and lo
                # passes accumulate into the same PSUM group.
                o_sb = outp.tile([P, H], dt.float32, tag="o_sb", name=f"osb{i}")
                npair = NQT // 2
                for nh in range(H // QC):
                    acc = ps_pool.tile([P, QC], dt.float32, tag="ps",
                                       name=f"m3_{i}_{nh}")
                    n = 0
                    for tp_i in range(npair):
                        sl = slice(2 * tp_i, 2 * tp_i + 2)
                        for qsrc in (q8h, q8l):
                            nc.tensor.matmul(
                                acc[:],
                                pt8[:, sl, :],
                                qsrc[:, sl, nh * QC:(nh + 1) * QC],
                                start=(n == 0),
                                stop=(n == 2 * npair - 1),
                                perf_mode=mybir.MatmulPerfMode.DoubleRow,
                            )
                            n += 1
                    # 1/sum scale on ScalarE (Identity supports AP scale)
                    nc.scalar.activation(
                        o_sb[:, nh * QC:(nh + 1) * QC], acc[:], AF.Identity,
                        scale=rinv[:],
                    )
                    # store each half as soon as it is scaled so the last
                    # iteration's store overlaps the second half's work
                    nc.sync.dma_start(
                        o_d[i * P:(i + 1) * P, nh * QC:(nh + 1) * QC],
                        o_sb[:, nh * QC:(nh + 1) * QC],
                    )

            at_cur = a0t_r
            mm3_prev = None

            for i in range(NAT):
                # MM2 nq-outer: each GT chunk finishes early so its
                # reduce_max overlaps the next chunk's matmuls.
                gt = []
                gmax = redp.tile([P, NQC], dt.float32, tag="gmax")
                for nq in range(NQC):
                    g = ps_pool.tile([P, QC], dt.float32, tag="ps",
                                     name=f"gt{nq}")
                    for k in range(KO):
                        nc.tensor.matmul(
                            g[:],
                            at_cur[:, k, :],
                            qwt_r[:, k, nq * QC:(nq + 1) * QC],
                            start=(k == 0),
                            stop=(k == KO - 1),
                        )
                    nc.vector.reduce_max(gmax[:, nq:nq + 1], g[:], axis=AX.X)
                    gt.append(g)

                negm = redp.tile([P, 1], dt.float32, tag="negm")
                nc.vector.reduce_max(negm[:], gmax[:], axis=AX.X, negate=True)

                # exps first so they're ahead of MM3's scales on ACT's
                # in-order queue
                p8_sb = ppool.tile([P, LQ], dt.float8e4, tag="p8_sb")
                sums = redp.tile([P, NQC], dt.float32, tag="sums")
                for nq in range(NQC):
                    nc.scalar.activation(
                        p8_sb[:, nq * QC:(nq + 1) * QC],
                        gt[nq][:],
                        AF.Exp,
                        bias=negm[:],
                        scale=1.0,
                        accum_out=sums[:, nq:nq + 1],
                    )
                sall = redp.tile([P, 1], dt.float32, tag="sall")
                nc.vector.reduce_sum(sall[:], sums[:], axis=AX.X)
                rinv = redp.tile([P, 1], dt.float32, tag="rinv")
                nc.vector.reciprocal(rinv[:], sall[:])

                # PE work that needs no softmax results fills the window
                # while ACT runs the exps: next a-tile's transposes, then
                # the previous iteration's MM3.
                if i + 1 < NAT:
                    at_next = a1t_r if i == 0 else prep_a_tile(i + 1)
                if mm3_prev is not None:
                    do_mm3(*mm3_prev)

                # transpose E=[a,q] -> ET=[q,a] on the PE (fp8, 1 cy/row;
                # the xbar DMA transpose is 2-byte-only) with strided DVE
                # evacuation. These ride the PE stream right behind MM2,
                # so no cross-engine chain blocks the sequencer. fp8
                # transpose mode requires an output element step of 2:
                # write each transposed block into the even lanes of a
                # [*, 2]-padded PSUM tile, compact in the DVE evacuation.
                pt8 = ptpool.tile([P, NQT, P], dt.float8e4, tag="pt8")
                if i == NAT - 1:
                    # last iteration: per-chunk transpose+evac directly
                    # behind each exp, shortening the serial tail
                    nt = NQT // NQC
                    for nq in range(NQC):
                        tp8c = tp_pool.tile([P, nt, P, 2], dt.float8e4,
                                            tag="tp", name=f"tp8c{nq}")
                        for t in range(nt):
                            tq = nq * nt + t
                            nc.tensor.transpose(
                                tp8c[:, t, :, 0],
                                p8_sb[:, tq * P:(tq + 1) * P],
                                id_f8[:],
                            )
                        nc.vector.tensor_copy(
                            pt8[:, nq * nt:(nq + 1) * nt, :],
                            tp8c[:, :, :, 0],
                        )
                else:
                    tp8 = tp_pool.tile([P, NQT, P, 2], dt.float8e4, tag="tp")
                    for t in range(NQT):
                        nc.tensor.transpose(
                            tp8[:, t, :, 0],
                            p8_sb[:, t * P:(t + 1) * P],
                            id_f8[:],
                        )
                    nc.vector.tensor_copy(pt8[:], tp8[:, :, :, 0])

                mm3_prev = (pt8, rinv, i)
                if i + 1 < NAT:
                    at_cur = at_next

            do_mm3(*mm3_prev)


_CACHE = {}


def build_nc():
    if "nc" in _CACHE:
        return _CACHE["nc"]
    nc = bacc.Bacc("TRN2", target_bir_lowering=False, debug=False)
    q_d = nc.dram_tensor("q", [LQ, H], dt.float32, kind="ExternalInput").ap()
    a_d = nc.dram_tensor("a", [LA, H], dt.float32, kind="ExternalInput").ap()
    w_d = nc.dram_tensor("w", [H, H], dt.float32, kind="ExternalInput").ap()
    b_d = nc.dram_tensor("b", [H], dt.float32, kind="ExternalInput").ap()
    o_d = nc.dram_tensor("o", [LA, H], dt.float32, kind="ExternalOutput").ap()
    with tile.TileContext(nc) as tc:
        _trace_kernel(tc, q_d, a_d, w_d, b_d, o_d)
    nc.compile()
    _CACHE["nc"] = nc
    return nc


def get_runner():
    """Build (once) a cached jitted SPMD executable over the 8 cores.

    Mirrors bass2jax.run_bass_via_pjrt's multi-core path, but caches the
    jitted callable so repeated invocations don't recompile.
    """
    if "runner" in _CACHE:
        return _CACHE["runner"]
    import jax
    from jax.sharding import Mesh, PartitionSpec
    from jax.experimental.shard_map import shard_map

    from concourse import bass2jax

    nc = build_nc()
    bass2jax.install_neuronx_cc_hook()

    partition_name = nc.partition_id_tensor.name if nc.partition_id_tensor else None
    in_names, out_names, out_avals, zero_outs = [], [], [], []
    for alloc in nc.m.functions[0].allocations:
        if not isinstance(alloc, mybir.MemoryLocationSet):
            continue
        name = alloc.memorylocations[0].name
        if alloc.kind == "ExternalInput":
            if name != partition_name:
                in_names.append(name)
        elif alloc.kind == "ExternalOutput":
            shape = tuple(alloc.tensor_shape)
            dtype = mybir.dt.np(alloc.dtype)
            out_names.append(name)
            out_avals.append(jax.core.ShapedArray(shape, dtype))
            zero_outs.append(np.zeros(shape, dtype))
    n_params = len(in_names)
    all_in_names = list(in_names) + list(out_names)
    if partition_name is not None:
        all_in_names.append(partition_name)

    def _body(*args):
        operands = list(args)
        if partition_name is not None:
            operands.append(bass2jax.partition_id_tensor())
        outs = bass2jax._bass_exec_p.bind(
            *operands,
            out_avals=tuple(out_avals),
            in_names=tuple(all_in_names),
            out_names=tuple(out_names),
            lowering_input_output_aliases=(),
            sim_require_finite=True,
            sim_require_nnan=True,
            nc=nc,
        )
        return tuple(outs)

    devices = jax.devices()[:B]
    mesh = Mesh(np.asarray(devices), ("core",))
    n_outs = len(out_names)
    in_specs = (PartitionSpec("core"),) * (n_params + n_outs)
    out_specs = (PartitionSpec("core"),) * n_outs
    sharded = jax.jit(
        shard_map(
            _body, mesh=mesh, in_specs=in_specs, out_specs=out_specs, check_rep=False
        ),
        keep_unused=True,
    )
    runner = (sharded, in_names, out_names, out_avals, zero_outs)
    _CACHE["runner"] = runner
    return runner


def run_cores(in_maps):
    """Run the kernel SPMD over 8 cores; in_maps is a list of 8 dicts."""
    sharded, in_names, out_names, out_avals, zero_outs = get_runner()
    concat_in = [
        np.concatenate([np.asarray(m[name]) for m in in_maps], axis=0)
        for name in in_names
    ]
    concat_zeros = [
        np.zeros((B * z.shape[0], *z.shape[1:]), z.dtype) for z in zero_outs
    ]
    out_arrs = sharded(*concat_in, *concat_zeros)
    return [
        {
            name: np.asarray(out_arrs[j]).reshape(B, *out_avals[j].shape)[c]
            for j, name in enumerate(out_names)
        }
        for c in range(B)
    ]


def kernel(q, a, w, b):
    q = np.ascontiguousarray(np.asarray(q, dtype=np.float32))
    a = np.ascontiguousarray(np.asarray(a, dtype=np.float32))
    w = np.ascontiguousarray(np.asarray(w, dtype=np.float32))
    b = np.ascontiguousarray(np.asarray(b, dtype=np.float32))
    assert q.shape == (B, LQ, H) and a.shape == (B, LA, H)
    assert w.shape == (H, H) and b.shape == (H,)

    in_maps = [{"q": q[i], "a": a[i], "w": w, "b": b} for i in range(B)]
    try:
        from concourse.bass_utils import run_bass_kernel_spmd

        results = run_bass_kernel_spmd(
            build_nc(), in_maps, core_ids=list(range(B))
        ).results
    except Exception:
        # fallback: cached jitted shard_map runner (same execution path)
        results = run_cores(in_maps)
    return np.stack([results[i]["o"] for i in range(B)], axis=0)
